# revision 9
# baseline (speedup 1.0000x reference)
"""Trainium2 Bass kernel v2 for 2-layer multi-head GAT (nn_GAT_38551626449703).

Design (8 NeuronCores, SPMD, one shared program):
  - Core k owns nodes [k*NPC, (k+1)*NPC).  Within a core, owned nodes are
    PERMUTED by out-degree (descending) and windowed 128 at a time; the
    shared per-window group count G_w = max over cores of the window's max
    degree (3-4% slot padding).  Host un-permutes the output rows.
  - Edge slots: window w, partition p, group g holds the g-th edge of the
    window's p-th node, so PARTITION p == SRC NODE p.  No one-hot matrix,
    no src gather: per-node src factors broadcast along the free dim.
  - exp/LeakyReLU factorization (exact, since exp is monotonic):
        ex = exp(lrelu(s1+s2)) = max(p1[src]*q2[dst], p1'[src]*q2'[dst])
    with p1=exp(s1), p1'=exp(.2*s1) per src node (SBUF-resident) and
    q2=exp(s2), q2'=exp(.2*s2) stored per dst node in the gather tables.
  - Gather tables (row dtype bf16; the f32 q-pairs live in the row pad):
      TW  [N2+1, 384]: [Wh (o,h)-order 256 | q2 f32x4 | q2' f32x4 | pad]
                        768B rows; row N2 is an all-zero sentinel.
      T2M [N+1, 128]:  [Wh2 64 | q2o f32 | q2o' f32 | pad] 256B rows;
                        rows are permuted-block (owner*NPC + rank);
                        row N is the sentinel.
    One dma_gather per window per layer: int16 indices with table base at
    row RB=32768 cover all rows via SIGNED offsets (verified on HW).
    Pad slots gather the sentinel row (q2=0 -> ex==0 exactly).
  - Accumulation: per group g one matmul with a STATIONARY bf16 identity,
    rhs = [Wh*ex | ex] -> PSUM f32 [128, 260] accumulates numerator+denom.
  - Wh columns are stored in (o,h) order so the ex broadcast multiplies
    keep the last AP dim packed (DVE 2x mode).
  - Between layers only the compact T2M shard (1.6 MB bf16) is AllGathered.
"""

import os
import sys

import numpy as np

sys.path.insert(0, "/opt/trn_rl_repo")

import concourse.bacc as bacc  # noqa: E402
import concourse.tile as tile  # noqa: E402
from concourse import mybir  # noqa: E402
from concourse.masks import make_identity  # noqa: E402

F32 = mybir.dt.float32
BF16 = mybir.dt.bfloat16
I16 = mybir.dt.int16
AF = mybir.ActivationFunctionType
ALU = mybir.AluOpType

# Problem constants
N = 50000
E = 800000
F_IN = 128
HID = 64
HEADS = 4
OUT = 64
ALPHA = 0.2
CORES = 8
NPC = N // CORES  # 6250
NW = (NPC + 127) // 128  # 49

RB = 32768  # gather table base row (int16 signed-offset trick)
GSPLIT = 7  # max groups per dma_gather call (7*128+16=912 <= 1024-desc ring)
RW1 = 384  # TW row, bf16 elems (768B)
RW2 = 128  # T2M row, bf16 elems (256B)
ACHUNK = 2048  # phase-A nodes per chunk
N2 = ((N + ACHUNK - 1) // ACHUNK) * ACHUNK  # 51200; rows N..N2-1 zero-x junk
NT1 = N2 // ACHUNK  # 25
SENT1 = N2  # TW sentinel row
SENT2 = N  # T2M sentinel row

NSWQ = int(os.environ.get("GAT_NSWQ", "4"))  # SWDGE queues

# Tile assigns the 8 DMASW completion-sem lanes round-robin over Pool-engine
# DMAs regardless of SWDGE queue, but a lane must stay on ONE queue (ucode
# constraint).  With NSWQ > 1 we partition the lanes per queue.
if NSWQ > 1:
    import concourse.bass_isa as _bass_isa
    import concourse.tile_sem_assignment as _tsa

    _orig_assign_tick = _tsa.TileClockTick._assign_tick

    def _lane_partitioned_assign_tick(self, inst):
        if (
            isinstance(inst, _tsa.DMAInst)
            and inst.engine == mybir.EngineType.Pool
            and not isinstance(inst, _bass_isa.UserSyncedRemoteDMADescs)
        ):
            qn = getattr(inst, "queue_num", 0) or 0
            per = getattr(self, "_q_lane_ctr", None)
            if per is None:
                per = self._q_lane_ctr = {}
            lanes = 8 // NSWQ
            c = per.get(qn, 0)
            per[qn] = c + 1
            self.next_sw_dma_idx = qn * lanes + (c % lanes)
        return _orig_assign_tick(self, inst)

    if _tsa.TileClockTick._assign_tick is not _lane_partitioned_assign_tick:
        _tsa.TileClockTick._assign_tick = _lane_partitioned_assign_tick


class Cfg:
    def __init__(self, gs):
        self.gs = tuple(int(g) for g in gs)  # per-window group counts
        assert len(self.gs) == NW
        self.sg = sum(self.gs)
        self.goff = np.concatenate([[0], np.cumsum(self.gs)]).astype(int)
        self.gmax = max(self.gs)
        # per-window gather calls: (s0, sq, idx col offset).  Each call's
        # index list gets 16 appended sentinel slots (idx >= 0) because the
        # ucode SKIPS a trailing run of negative indices (HW-probed); the
        # junk slots land in group s0+sq, partitions 0..15.
        self.calls = []
        off = 0
        for g in self.gs:
            wcalls = []
            for s0 in range(0, g, GSPLIT):
                sq = min(GSPLIT, g - s0)
                wcalls.append((s0, sq, off))
                off += sq * 8 + 1
            self.calls.append(wcalls)
        self.icols = off


def _elu_bf(nc, pool, x_ap, cols, out_ap, tag, dt=BF16):
    """out = elu(x) = relu(x) + exp(x - relu(x)) - 1.  relu/exp on ACT."""
    rl = pool.tile([128, cols], dt, tag=f"{tag}_rl")
    nc.scalar.activation(rl[:], x_ap, AF.Relu)
    t = pool.tile([128, cols], dt, tag=f"{tag}_t")
    nc.vector.tensor_tensor(t[:], x_ap, rl[:], op=ALU.subtract)
    nc.scalar.activation(t[:], t[:], AF.Exp)
    # out = (rl + (-1)) + t
    nc.vector.scalar_tensor_tensor(
        out=out_ap, in0=rl[:], scalar=-1.0, in1=t[:], op0=ALU.add, op1=ALU.add
    )


def build_nc(cfg: Cfg, reps=None, sim_collective: bool = False):
    reps = reps or {}
    phases = os.environ.get("GAT_PHASES", "ABCDE")
    gs, goff = cfg.gs, cfg.goff
    SG8 = 8 * cfg.sg
    GMAX = cfg.gmax

    # One dma_gather call's descriptors must fit the SWDGE ring (default
    # dynamic_dma_scratch 16384B = 1024 descriptors); GSPLIT=6 groups
    # (768+16 descriptors) is the HW-proven safe call size.
    nc = bacc.Bacc(
        "TRN2",
        target_bir_lowering=False,
        debug=False,
        num_swdge_queues=NSWQ,
    )

    # ---- external I/O ----
    xT_ext = nc.dram_tensor("xT", [F_IN, N2], BF16, kind="ExternalInput")
    xTo_ext = nc.dram_tensor("xTown", [F_IN, NW * 128], BF16, kind="ExternalInput")
    wext_ext = nc.dram_tensor("wext", [F_IN, 272], BF16, kind="ExternalInput")
    w2ext_ext = nc.dram_tensor("w2ext", [2, 128, 68], BF16, kind="ExternalInput")
    idx1_ext = nc.dram_tensor("idx1", [128, cfg.icols], I16, kind="ExternalInput")
    idx2_ext = nc.dram_tensor("idx2", [128, cfg.icols], I16, kind="ExternalInput")
    out_ext = nc.dram_tensor("out", [NPC, OUT], F32, kind="ExternalOutput")

    # ---- internal DRAM ----
    tw = nc.dram_tensor("TW", [N2 + 1, RW1], BF16)
    t2msh = nc.dram_tensor("T2Msh", [NPC, RW2], BF16)
    if CORES > 1 and not sim_collective:
        t2m = nc.dram_tensor("T2M", [N + 1, RW2], BF16, addr_space="Shared")
    else:
        t2m = nc.dram_tensor("T2M", [N + 1, RW2], BF16)

    def q_of(w):
        return w % NSWQ

    with tile.TileContext(nc) as tc, tc.tile_pool(name="const", bufs=1) as cpool:
        # ======== persistent SBUF ========
        ident = cpool.tile([128, 128], F32)
        make_identity(nc, ident[:])
        identb = cpool.tile([128, 128], BF16)
        nc.vector.tensor_copy(identb[:], ident[:])
        wextsb = cpool.tile([F_IN, 272], BF16)
        nc.sync.dma_start(wextsb[:], wext_ext[:])
        w2sb = cpool.tile([128, 2 * 68], BF16)
        nc.sync.dma_start(w2sb[:, 0:68], w2ext_ext[0])
        nc.sync.dma_start(w2sb[:, 68:136], w2ext_ext[1])
        i16_1 = cpool.tile([128, cfg.icols], I16)
        nc.sync.dma_start(i16_1[:], idx1_ext[:])
        i16_2 = cpool.tile([128, cfg.icols], I16)
        nc.sync.dma_start(i16_2[:], idx2_ext[:])
        p1sb = cpool.tile([128, NW * 8], F32)
        p1osb = cpool.tile([128, NW * 2], F32)
        hcat = cpool.tile([128, NW * 256], BF16)
        out_all = cpool.tile([128, NW * OUT], F32)
        xo_all = cpool.tile([128, NW * 128], BF16)
        nc.sync.dma_start(xo_all[:], xTo_ext[:])

        # sentinel rows (all zeros)
        z1 = cpool.tile([1, RW1], BF16)
        nc.vector.memset(z1[:], 0.0)
        nc.sync.dma_start(tw[SENT1 : SENT1 + 1, :], z1[:])
        z2 = cpool.tile([1, RW2], BF16)
        nc.vector.memset(z2[:], 0.0)
        nc.sync.dma_start(t2m[SENT2 : SENT2 + 1, :], z2[:])

        # ======== phase A: build TW (all nodes) + p1/p1' (own nodes) ======
        with (
            tc.tile_pool(name="psA", bufs=2, space="PSUM") as psA,
            tc.tile_pool(name="sbA", bufs=3) as sbA,
        ):
            for _ra in range(reps.get("A", 1) if "A" in phases else 0):
                for t in range(NT1):
                    c0 = ACHUNK * t
                    xt = sbA.tile([128, ACHUNK], BF16, tag="xt")
                    nc.sync.dma_start(xt[:], xT_ext[:, c0 : c0 + ACHUNK])
                    wt = sbA.tile([128, (ACHUNK // 128) * 272], BF16, tag="wt")
                    ps_q = psA.tile([128, (ACHUNK // 128) * 8], F32, tag="ps_q")
                    qv = wt[:].rearrange("p (k e) -> p k e", e=272)
                    for par in range(ACHUNK // 256):
                        ps_wh = psA.tile([128, 512], F32, tag="ps_wh")
                        for h in range(2):
                            q = 2 * par + h
                            nc.tensor.matmul(
                                ps_wh[:, 256 * h : 256 * (h + 1)],
                                xt[:, 128 * q : 128 * (q + 1)],
                                wextsb[:, 0:256],
                                start=True,
                                stop=True,
                            )
                            nc.tensor.matmul(
                                ps_q[:, 8 * q : 8 * (q + 1)],
                                xt[:, 128 * q : 128 * (q + 1)],
                                wextsb[:, 256:264],
                                start=True,
                                stop=True,
                            )
                        # convert Wh pair to bf16 on ACT (DVE is the
                        # busier engine overall; ACT idles in phase A)
                        for h2 in range(2):
                            q = 2 * par + h2
                            nc.scalar.copy(
                                wt[:, 272 * q + 16 : 272 * (q + 1)],
                                ps_wh[:, 256 * h2 : 256 * (h2 + 1)],
                            )
                    nc.scalar.activation(
                        qv[:, :, 0:16].bitcast(F32), ps_q[:], AF.Exp
                    )
                    nc.sync.dma_start(
                        tw[c0 : c0 + ACHUNK, 0:272].rearrange(
                            "(k p) e -> p k e", p=128
                        ),
                        wt[:].rearrange("p (k e) -> p k e", e=272),
                    )
                # p1/p1' for own (permuted) nodes
                for w in range(NW):
                    ps_p = psA.tile([128, 8], F32, tag="ps_p")
                    nc.tensor.matmul(
                        ps_p[:],
                        xo_all[:, 128 * w : 128 * (w + 1)],
                        wextsb[:, 264:272],
                        start=True,
                        stop=True,
                    )
                    nc.scalar.activation(
                        p1sb[:, 8 * w : 8 * (w + 1)], ps_p[:], AF.Exp
                    )

        # ======== phase B: layer-1 edge processing ========
        with (
            tc.tile_pool(name="psB", bufs=2, space="PSUM") as psB,
            tc.tile_pool(name="sbB", bufs=3) as sbB,
        ):
            for _rb in range(reps.get("B", 1) if "B" in phases else 0):
                for w in range(NW):
                    G = gs[w]
                    wn = min(128, NPC - 128 * w)
                    # each call gets a DISJOINT span [s0+i, s0+i+sq+1) in the
                    # tile: its 16 sentinel-junk slots land in a dead gap
                    # group, so calls neither serialize nor corrupt each other
                    ncall = len(cfg.calls[w])
                    g1 = sbB.tile([128, (GMAX + 6) * RW1], BF16, tag="g1")
                    gx = g1[:].rearrange("p (g c) -> p g c", c=RW1)
                    tab = sbB.tile([128, GMAX * 8], F32, tag="tab")
                    ex = sbB.tile([128, GMAX * 4], BF16, tag="ex")
                    ex3 = ex[:, : G * 4].rearrange("p (g h) -> p g h", h=4)
                    for i, (s0, sq, ioff) in enumerate(cfg.calls[w]):
                        nc.gpsimd.dma_gather(
                            gx[:, s0 + i : s0 + i + sq + 1, :],
                            tw[RB:, :],
                            i16_1[:, ioff : ioff + sq * 8 + 1],
                            sq * 128 + 16,
                            sq * 128 + 16,
                            RW1,
                            queue_num=(w + i) % NSWQ,
                        )
                    for i, (s0, sq, ioff) in enumerate(cfg.calls[w]):
                        sp = gx[:, s0 + i : s0 + i + sq, :]
                        nc.vector.tensor_tensor(
                            tab[:, 8 * s0 : 8 * (s0 + sq)].rearrange(
                                "p (g c) -> p g c", c=8
                            ),
                            sp[:, :, 0:16].bitcast(F32),
                            p1sb[:, 8 * w : 8 * (w + 1)]
                            .unsqueeze(1)
                            .to_broadcast([128, sq, 8]),
                            op=ALU.mult,
                        )
                    t3 = tab[:, : G * 8].rearrange("p (g c) -> p g c", c=8)
                    nc.vector.tensor_tensor(
                        ex3, t3[:, :, 0:4], t3[:, :, 4:8], op=ALU.max
                    )
                    # scale Wh by ex IN PLACE per call span
                    for i, (s0, sq, ioff) in enumerate(cfg.calls[w]):
                        sp = gx[:, s0 + i : s0 + i + sq, :]
                        nc.vector.tensor_tensor(
                            sp[:, :, 16:272].rearrange("p g (o h) -> p g o h", h=4),
                            sp[:, :, 16:272].rearrange("p g (o h) -> p g o h", h=4),
                            ex3[:, s0 : s0 + sq, :]
                            .unsqueeze(2)
                            .to_broadcast([128, sq, 64, 4]),
                            op=ALU.mult,
                        )
                    # numerator over groups (identity stationary)
                    ps_u = psB.tile([128, 256], F32, tag="ps_u")
                    for g in range(G):
                        tg = g + g // GSPLIT  # tile group with gap shift
                        nc.tensor.matmul(
                            ps_u[:],
                            identb[:],
                            g1[:, RW1 * tg + 16 : RW1 * tg + 272],
                            start=(g == 0),
                            stop=(g == G - 1),
                        )
                    # denominator: free-dim reduce of ex over groups
                    den = sbB.tile([128, 4], F32, tag="den")
                    nc.vector.tensor_reduce(
                        den[:].unsqueeze(2),
                        ex[:, : G * 4].rearrange("p (g h) -> p h g", h=4),
                        mybir.AxisListType.X,
                        ALU.add,
                    )
                    nc.vector.tensor_scalar_add(den[:], den[:], 1e-30)
                    nc.vector.reciprocal(den[:], den[:])
                    hp = sbB.tile([128, 256], BF16, tag="hp")
                    nc.vector.tensor_tensor(
                        hp[:].rearrange("p (o h) -> p o h", h=4),
                        ps_u[:, 0:256].rearrange("p (o h) -> p o h", h=4),
                        den[:].unsqueeze(1).to_broadcast([128, 64, 4]),
                        op=ALU.mult,
                    )
                    _elu_bf(
                        nc, sbB, hp[:], 256, hcat[:, 256 * w : 256 * (w + 1)], "e1"
                    )
                    # ---- fused phase C: this window's T2M shard rows ----
                    ps2 = psB.tile([128, 68], F32, tag="ps2")
                    for c in range(2):
                        ps_t = psB.tile([128, 128], BF16, tag="ps_t")
                        nc.tensor.transpose(
                            ps_t[:],
                            hcat[:, 256 * w + 128 * c : 256 * w + 128 * (c + 1)],
                            identb[:],
                        )
                        hT = sbB.tile([128, 128], BF16, tag="hT")
                        nc.scalar.copy(hT[:], ps_t[:])
                        nc.tensor.matmul(
                            ps2[:],
                            hT[:],
                            w2sb[:, 68 * c : 68 * (c + 1)],
                            start=(c == 0),
                            stop=(c == 1),
                        )
                    row = sbB.tile([128, 68], BF16, tag="row")
                    nc.scalar.copy(row[:, 0:64], ps2[:, 0:64])
                    nc.scalar.activation(
                        row[:, 64:68].bitcast(F32), ps2[:, 64:66], AF.Exp
                    )
                    nc.scalar.activation(
                        p1osb[:, 2 * w : 2 * (w + 1)], ps2[:, 66:68], AF.Exp
                    )
                    nc.sync.dma_start(
                        t2msh[128 * w : 128 * w + wn, 0:68], row[:wn, :]
                    )

        # ======== phase D: allgather T2M ========
        for _rd in range(reps.get("D", 1) if "D" in phases else 0):
            if sim_collective:
                nc.sync.dma_start(t2m[0:NPC, :], t2msh[:])
            elif CORES > 1:
                nc.gpsimd.collective_compute(
                    "AllGather",
                    ALU.bypass,
                    replica_groups=[list(range(CORES))],
                    ins=[t2msh[:]],
                    outs=[t2m[0:N, :]],
                )
            else:
                nc.sync.dma_start(t2m[0:N, :], t2msh[:])

        # ======== phase E: layer-2 edge processing ========
        with (
            tc.tile_pool(name="psE", bufs=4, space="PSUM") as psE,
            tc.tile_pool(name="sbE", bufs=3) as sbE,
        ):
            for _re in range(reps.get("E", 1) if "E" in phases else 0):
                for w in range(NW):
                    G = gs[w]
                    wn = min(128, NPC - 128 * w)
                    ncall = len(cfg.calls[w])
                    g2 = sbE.tile([128, (GMAX + 6) * RW2], BF16, tag="g2")
                    gx = g2[:].rearrange("p (g c) -> p g c", c=RW2)
                    tab = sbE.tile([128, GMAX * 2], F32, tag="tab2")
                    ex = sbE.tile([128, GMAX], BF16, tag="ex2")
                    for i, (s0, sq, ioff) in enumerate(cfg.calls[w]):
                        nc.gpsimd.dma_gather(
                            gx[:, s0 + i : s0 + i + sq + 1, :],
                            t2m[RB:, :],
                            i16_2[:, ioff : ioff + sq * 8 + 1],
                            sq * 128 + 16,
                            sq * 128 + 16,
                            RW2,
                            queue_num=(w + i) % NSWQ,
                        )
                    for i, (s0, sq, ioff) in enumerate(cfg.calls[w]):
                        sp = gx[:, s0 + i : s0 + i + sq, :]
                        nc.vector.tensor_tensor(
                            tab[:, 2 * s0 : 2 * (s0 + sq)].rearrange(
                                "p (g c) -> p g c", c=2
                            ),
                            sp[:, :, 64:68].bitcast(F32),
                            p1osb[:, 2 * w : 2 * (w + 1)]
                            .unsqueeze(1)
                            .to_broadcast([128, sq, 2]),
                            op=ALU.mult,
                        )
                    t3 = tab[:, : G * 2].rearrange("p (g c) -> p g c", c=2)
                    nc.vector.tensor_tensor(
                        ex[:, :G].unsqueeze(2),
                        t3[:, :, 0:1],
                        t3[:, :, 1:2],
                        op=ALU.max,
                    )
                    for i, (s0, sq, ioff) in enumerate(cfg.calls[w]):
                        sp = gx[:, s0 + i : s0 + i + sq, :]
                        nc.vector.tensor_tensor(
                            sp[:, :, 0:64],
                            sp[:, :, 0:64],
                            ex[:, s0 : s0 + sq]
                            .unsqueeze(2)
                            .to_broadcast([128, sq, 64]),
                            op=ALU.mult,
                        )
                    ps_u = psE.tile([128, 64], F32, tag="ps_u2")
                    for g in range(G):
                        tg = g + g // GSPLIT
                        nc.tensor.matmul(
                            ps_u[:],
                            identb[:],
                            g2[:, RW2 * tg : RW2 * tg + 64],
                            start=(g == 0),
                            stop=(g == G - 1),
                        )
                    den = sbE.tile([128, 1], F32, tag="den2")
                    nc.vector.tensor_reduce(
                        den[:].unsqueeze(2),
                        ex[:, :G].unsqueeze(1),
                        mybir.AxisListType.X,
                        ALU.add,
                    )
                    nc.vector.tensor_scalar_add(den[:], den[:], 1e-30)
                    nc.vector.reciprocal(den[:], den[:])
                    op_t = sbE.tile([128, OUT], F32, tag="op_t")
                    nc.vector.tensor_tensor(
                        op_t[:],
                        ps_u[:, 0:64],
                        den[:].to_broadcast([128, 64]),
                        op=ALU.mult,
                    )
                    _elu_bf(
                        nc, sbE, op_t[:], OUT,
                        out_all[:, OUT * w : OUT * (w + 1)], "e2", dt=F32,
                    )
                # batched output write (full windows, then the ragged tail)
                nfull = NPC // 128  # 48
                nc.sync.dma_start(
                    out_ext[0 : 128 * nfull, :].rearrange(
                        "(k p) e -> p k e", p=128
                    ),
                    out_all[:, : nfull * OUT].rearrange(
                        "p (k e) -> p k e", e=OUT
                    ),
                )
                nc.sync.dma_start(
                    out_ext[128 * nfull : NPC, :],
                    out_all[: NPC - 128 * nfull, nfull * OUT :],
                )

    nc.compile()
    return nc


# ---------------------------------------------------------------------------
# Host-side preparation and execution
# ---------------------------------------------------------------------------


def _perms_and_schedule(edges):
    src = np.asarray(edges[0], dtype=np.int64)
    deg = np.bincount(src, minlength=N)
    perms, ranks = [], []
    gw = np.zeros((CORES, NW), dtype=np.int64)
    last = np.zeros((CORES, NW), dtype=np.int64)
    for k in range(CORES):
        d = deg[k * NPC : (k + 1) * NPC]
        perm = np.argsort(-d, kind="stable")
        rank = np.empty(NPC, dtype=np.int64)
        rank[perm] = np.arange(NPC)
        perms.append(perm)
        ranks.append(rank)
        ds = np.pad(d[perm], (0, NW * 128 - NPC)).reshape(NW, 128)
        gw[k] = ds.max(axis=1)
        last[k] = ds[:, 127]
    g = gw.max(axis=0)
    g = g + (last.max(axis=0) == g)  # force last linear slot to be a pad
    g = np.maximum(g, 1)
    return perms, ranks, Cfg(g)


def make_cfg(edges):
    return _perms_and_schedule(edges)[2]


def _pack16(vals):
    """[G*128] linear slot values -> [128, G*8] int16 (16-wrap, 8 replicas)."""
    g8 = len(vals) // 16
    w = vals.reshape(g8, 16).T  # [16, G*8]
    return np.tile(w, (8, 1)).astype(np.int16)


def prepare_inputs(cfg: Cfg, x, edges, W_heads, a_heads, W_out, a_out):
    import ml_dtypes

    bf16 = ml_dtypes.bfloat16
    src = np.asarray(edges[0], dtype=np.int64)
    dst = np.asarray(edges[1], dtype=np.int64)
    x = np.asarray(x, np.float32)
    Wh = np.asarray(W_heads, np.float32)
    ah = np.asarray(a_heads, np.float32)
    Wo = np.asarray(W_out, np.float32)
    ao = np.asarray(a_out, np.float32)

    perms, ranks, _ = _perms_and_schedule(edges)

    # wext: [Wh (o,h)-order 256 | c2 4 | .2*c2 | c1 4 | .2*c1]
    wext = np.zeros((F_IN, 272), np.float32)
    for h in range(HEADS):
        wext[:, np.arange(HID) * 4 + h] = Wh[h]  # col o*4+h = Wh[h][:, o]
    c1 = np.stack([Wh[h] @ ah[h, :HID] for h in range(HEADS)], axis=1)
    c2 = np.stack([Wh[h] @ ah[h, HID:] for h in range(HEADS)], axis=1)
    wext[:, 256:260] = c2
    wext[:, 260:264] = ALPHA * c2
    wext[:, 264:268] = c1
    wext[:, 268:272] = ALPHA * c1

    # w2ext rows are hcat features in (o,h) order: row f=(o*4+h) = Wo[h*64+o]
    f = np.arange(256)
    Wop = Wo[(f % 4) * HID + (f // 4)]
    w2 = np.zeros((256, 68), np.float32)
    w2[:, 0:64] = Wop
    w2[:, 64] = Wop @ ao[OUT:]
    w2[:, 65] = ALPHA * (Wop @ ao[OUT:])
    w2[:, 66] = Wop @ ao[:OUT]
    w2[:, 67] = ALPHA * (Wop @ ao[:OUT])

    xT = np.zeros((F_IN, N2), np.float32)
    xT[:, :N] = x.T
    xT = xT.astype(bf16)

    # global layer-2 row of node v: owner*NPC + rank
    row2 = np.empty(N, dtype=np.int64)
    for k in range(CORES):
        row2[k * NPC : (k + 1) * NPC] = k * NPC + ranks[k]

    common = dict(
        wext=wext.astype(bf16),
        w2ext=np.ascontiguousarray(w2.reshape(2, 128, 68).astype(bf16)),
    )

    in_maps = []
    for k in range(CORES):
        own = (src >= k * NPC) & (src < (k + 1) * NPC)
        es = ranks[k][src[own] - k * NPC]  # rank 0..NPC-1
        ed = dst[own]
        order = np.argsort(es, kind="stable")
        es, ed = es[order], ed[order]
        counts = np.bincount(es, minlength=NW * 128)
        starts = np.concatenate([[0], np.cumsum(counts)])[:-1]
        g = np.arange(len(es)) - starts[es]
        w = es // 128
        p = es % 128
        pos = (cfg.goff[w] + g) * 128 + p
        flat1 = np.full(cfg.sg * 128, SENT1 - RB, dtype=np.int64)
        flat1[pos] = ed - RB
        flat2 = np.full(cfg.sg * 128, SENT2 - RB, dtype=np.int64)
        flat2[pos] = row2[ed] - RB
        def pack_calls(flat, sent):
            parts = []
            for w in range(NW):
                base = 128 * cfg.goff[w]
                for s0, sq, _ in cfg.calls[w]:
                    v = flat[base + 128 * s0 : base + 128 * (s0 + sq)]
                    v = np.concatenate([v, np.full(16, sent, np.int64)])
                    parts.append(_pack16(v))
            return np.concatenate(parts, axis=1)

        i1 = pack_calls(flat1, SENT1 - RB)
        i2 = pack_calls(flat2, SENT2 - RB)
        xo = np.zeros((F_IN, NW * 128), np.float32)
        xo[:, :NPC] = x.T[:, k * NPC + perms[k]]
        in_maps.append(
            dict(
                common,
                xT=xT,
                xTown=xo.astype(bf16),
                idx1=np.ascontiguousarray(i1),
                idx2=np.ascontiguousarray(i2),
            )
        )
    return in_maps, perms


_NC_CACHE = {}


def get_nc(cfg: Cfg):
    key = cfg.gs
    if key not in _NC_CACHE:
        _NC_CACHE[key] = build_nc(cfg)
    return _NC_CACHE[key]


def run(inputs, trace=False, **spmd_kwargs):
    from concourse.bass_utils import run_bass_kernel_spmd

    edges = np.asarray(inputs["edges"])
    cfg = make_cfg(edges)
    nc = get_nc(cfg)
    in_maps, perms = prepare_inputs(
        cfg,
        inputs["x"],
        edges,
        inputs["W_heads"],
        inputs["a_heads"],
        inputs["W_out"],
        inputs["a_out"],
    )
    res = run_bass_kernel_spmd(
        nc, in_maps, core_ids=list(range(CORES)), trace=trace, **spmd_kwargs
    )
    out = np.zeros((N, OUT), np.float32)
    for k in range(CORES):
        out[k * NPC + perms[k]] = res.results[k]["out"]
    return out, res


def kernel(**inputs):
    return run(inputs)[0]
